# revision 25
# baseline (speedup 1.0000x reference)
"""Multi-head self-attention (B=4, S=2048, D=768, H=12, dh=64) on 8 trn2 cores.

Sharding: core = b*2 + g  (b = batch 0..3, g = head-group of 6 heads).
Each core computes q/k/v projections for its 6 heads over the full sequence,
masked softmax attention, and a partial output projection (column slice of
o_w => row-parallel). Host sums the two partial outputs per batch element.

v2 design (single fused schedule, all engines near-balanced):
  - mask gather: only unmasked k positions (padded to a multiple of 128) are
    shipped/projected/exp'd; padding columns get a -1e30 per-partition bias
    inside the ACT exp instruction (out = exp(scale*s + bias)).
  - kT stored PAIRED: kTp[0:64, pair, :] = even head, kTp[64:128, pair, :] =
    odd head.  Score matmuls are K=64 row-tiled (tile_position rows {0,64})
    so the two heads of a pair run CONCURRENTLY on disjoint row-groups of
    the PE array -- no zero padding, half the score-matmul wall time.
  - exp is split between ACT and DVE: for kc in DVE_KC the "exp" is a single
    DVE tensor_scalar  t = s*23.0831 + (16256 + 2^23)  computed in fp32.
    Adding 2^23 forces round-to-integer; the low 16 bits of each f32 are
    then exactly the bf16 bit pattern of 2^(s*log2e/8) with a piecewise
    -linear mantissa (Schraudolph).  The ctx matmul reads those weights
    directly through a stride-2 bf16 bitcast view -- no second pass.
    The ACT path carries a +0.029807 bias (= half the max log-ratio of the
    linear approx) so both paths have the same mean; the residual +-3%
    sawtooth is random across kv and averages out in the ctx dot product.
  - ONE psum pool for everything (tags: "s" x2, "c" x2 = 8 banks), open for
    the whole program: projection chunks, score pairs, ctx accumulations and
    out-projection chunks all rotate through it, so there is no pool
    boundary and the phases overlap.  Projection / out-projection chunks are
    hand-interleaved into the attention emission (one chunk per kc slot) to
    fill the PE slack of the ACT/DVE-bound attention stream.
  - softmax denominators come free from an appended ones-column in v (psum
    row 64 of the ctx matmul).  Tail per (pair, qh): approx-reciprocal read
    straight from psum row 64, K=128 ones-matmul broadcast into psum rows
    64..127, one tensor_tensor multiply psum x psum -> ctx_sb (bf16).
  - biases: q/k bias via per-partition DVE tensor_scalar on psum eviction;
    v bias via contraction-augmentation (ones row in xvT, v_b row in wvT);
    o_b broadcast across partitions once, added on psum evict (zeros passed
    for the g==1 cores so the host sum applies it once).
"""

import numpy as np
import ml_dtypes

import concourse.bass as bass
import concourse.mybir as mybir
import concourse.tile as tile
from concourse import bacc
from concourse.bass_utils import run_bass_kernel_spmd

BS, SEQ, DIM, NH = 4, 2048, 768, 12
DH = 64
HEADS = 6            # heads per core
NPAIR = 3            # head pairs per core
DGRP = HEADS * DH    # 384
N_CORES = 8
P = 128
QH = 1024            # q-half width in the attention loop
KIN = DIM // P       # 6 contraction chunks for q/k proj
KIN_V = 7            # 768 inputs + ones row, padded to 896

F32 = mybir.dt.float32
BF16 = mybir.dt.bfloat16

MM_DT = BF16
MM_NP = ml_dtypes.bfloat16

NEG = -1.0e30
# ACT-path exp bias: centers the exact exp against the DVE piecewise-linear
# exp2 (max log2 ratio 0.08607 -> shift both means by half of that).
BIAS_CENTER = 0.0430365 * 0.6931471805599453   # = 0.0298296 (natural log)
# DVE Schraudolph constants: bf16 bits k = s * (128*log2e/8) + 16256,
# computed as  f32( s*23.08312 + (16256 + 2^23) )  -- the 2^23 add rounds
# k to the nearest integer in the f32 mantissa.
SCH_SCALE = 128.0 * 1.4426950408889634 / 8.0   # 23.083120654
SCH_BIAS = 16256.0 + 8388608.0

# kv chunks whose exp runs on DVE instead of ACT (never the last, padded one)
DVE_KC = (2, 5)


def _build(NKV: int):
    """Build the per-core Bass program, parameterized by padded kv length."""
    KC = NKV // P          # kv chunks
    dve_kc = set(c for c in DVE_KC if c < KC - 1)

    nc = bacc.Bacc(None, target_bir_lowering=False, debug=False)

    xqT = nc.declare_dram_parameter("xqT", [DIM, SEQ], MM_DT, isOutput=False)
    xkT = nc.declare_dram_parameter("xkT", [DIM, NKV], MM_DT, isOutput=False)
    xvT = nc.declare_dram_parameter("xvT", [P * KIN_V, NKV], MM_DT, isOutput=False)
    wqT = nc.declare_dram_parameter("wqT", [DIM, DGRP], MM_DT, isOutput=False)
    wkT = nc.declare_dram_parameter("wkT", [DIM, DGRP], MM_DT, isOutput=False)
    wvT = nc.declare_dram_parameter("wvT", [P * KIN_V, DGRP], MM_DT, isOutput=False)
    woT = nc.declare_dram_parameter("woT", [DGRP, DIM], MM_DT, isOutput=False)
    qb = nc.declare_dram_parameter("qb", [DGRP], F32, isOutput=False)
    kb = nc.declare_dram_parameter("kb", [DGRP], F32, isOutput=False)
    ob = nc.declare_dram_parameter("ob", [DIM], F32, isOutput=False)
    pb = nc.declare_dram_parameter("pb", [NKV], F32, isOutput=False)
    out = nc.declare_dram_parameter("out", [SEQ, DIM], F32, isOutput=True)

    xqT_r = xqT.rearrange("(kk pi) n -> pi kk n", pi=P)
    xkT_r = xkT.rearrange("(kk pi) n -> pi kk n", pi=P)
    xvT_r = xvT.rearrange("(kk pi) n -> pi kk n", pi=P)
    wqT_r = wqT.rearrange("(kk pi) n -> pi kk n", pi=P)
    wkT_r = wkT.rearrange("(kk pi) n -> pi kk n", pi=P)
    wvT_r = wvT.rearrange("(kk pi) n -> pi kk n", pi=P)
    woT_r = woT.rearrange("(kk pi) n -> pi kk n", pi=P)
    qb_r = qb.rearrange("(m pi) -> pi m", pi=P)
    kb_r = kb.rearrange("(m pi) -> pi m", pi=P)
    pb_r = pb.rearrange("(c pi) -> pi c", pi=P)

    # k-proj slices along kv
    ksl = []
    o = 0
    while o < NKV:
        w = min(512, NKV - o)
        ksl.append((o, w))
        o += w

    with tile.TileContext(nc) as tc:
        with (
            tc.tile_pool(name="const", bufs=1) as const,
            tc.tile_pool(name="persist", bufs=1) as persist,
            tc.tile_pool(name="expp", bufs=4) as expp,
            tc.tile_pool(name="schp", bufs=4) as schp,
            tc.tile_pool(name="outp", bufs=3) as outp,
            tc.tile_pool(name="ps", bufs=2, space="PSUM") as ps,
        ):
            # ---- constants (DMA order = consumer order) ----
            pb_sb = const.tile([P, KC], F32)
            nc.sync.dma_start(pb_sb[:], pb_r)
            qb_sb = const.tile([P, 3], F32)
            nc.sync.dma_start(qb_sb[:], qb_r)
            kb_sb = const.tile([P, 3], F32)
            nc.sync.dma_start(kb_sb[:], kb_r)
            ob_row = const.tile([P, DIM], F32)
            nc.vector.memset(ob_row[:], 0.0)
            nc.sync.dma_start(ob_row[0:1, :], ob[None, :])
            wq_sb = const.tile([P, KIN, DGRP], MM_DT)
            nc.sync.dma_start(wq_sb[:], wqT_r)

            # persistent inputs (DMA'd in slices, just-in-time order:
            # q nt0/nt1 + k slice 0 unblock the first score tile; xv arrives
            # in time for the first ctx matmuls; later xk/xq slices follow)
            xq_sb = persist.tile([P, KIN, SEQ], MM_DT)
            nc.sync.dma_start(xq_sb[:, :, 0:512], xqT_r[:, :, 0:512])
            nc.sync.dma_start(xq_sb[:, :, 512:1024], xqT_r[:, :, 512:1024])
            wk_sb = const.tile([P, KIN, DGRP], MM_DT)
            nc.sync.dma_start(wk_sb[:], wkT_r)
            xk_sb = persist.tile([P, KIN, NKV], MM_DT)
            o0, w = ksl[0]
            nc.sync.dma_start(xk_sb[:, :, o0:o0 + w], xkT_r[:, :, o0:o0 + w])
            wv_sb = const.tile([P, KIN_V, DGRP], MM_DT)
            nc.sync.dma_start(wv_sb[:], wvT_r)
            xv_sb = persist.tile([P, KIN_V, NKV], MM_DT)
            nc.sync.dma_start(xv_sb[:], xvT_r)
            for o0, w in ksl[1:]:
                nc.sync.dma_start(xk_sb[:, :, o0:o0 + w], xkT_r[:, :, o0:o0 + w])
            wo_sb = const.tile([P, 3, DIM], MM_DT)
            nc.sync.dma_start(wo_sb[:], woT_r)
            nc.sync.dma_start(xq_sb[:, :, 1024:1536], xqT_r[:, :, 1024:1536])
            nc.sync.dma_start(xq_sb[:, :, 1536:2048], xqT_r[:, :, 1536:2048])

            # ones-row matrix: row 0 all-ones, rows 1..127 zero. As lhsT this
            # replicates row 0 of the rhs into all M output partitions with a
            # full K=128 contraction (keeps the PE HAM clock warm).
            ones2_sb = const.tile([P, P], F32)
            nc.vector.memset(ones2_sb[:], 0.0)
            nc.vector.memset(ones2_sb[0:1, :], 1.0)
            ob_bc = const.tile([P, DIM], F32)
            # dummy exp: pulls the ~2.7us ACT_TABLE_LOAD into the DMA ramp
            warm_t = const.tile([P, 1], F32)
            nc.scalar.activation(warm_t[:], ones2_sb[:, 0:1],
                                 mybir.ActivationFunctionType.Exp)

            # ---- persistent activations ----
            qT_sb = persist.tile([P, NPAIR, SEQ], MM_DT)
            # kTz: per head h, half 64*(h%2) holds kT_h, other half zero.
            # (A K=64 row-tiled score matmul would halve this, but 64-mode
            # matmuls interleaved with an OPEN 128-mode ctx accumulation
            # group crash the device -- NRT_EXEC_UNIT_UNRECOVERABLE.)
            kTz_sb = persist.tile([P, HEADS, NKV], MM_DT)
            nc.gpsimd.memset(kTz_sb[:], 0.0)
            # v lanes are 128 wide per head: slot 0 = ones column (softmax
            # sums land in psum row 0, readable by the custom-DVE recip at
            # partition offset 0), slots 64..127 = the 64 v dims (ctx dims
            # land in psum rows 64..127, 64-aligned for the eviction copy),
            # slots 1..63 zero. PSUM/DVE partition offsets must be 0/64.
            v_sb = persist.tile([P, KC, HEADS * 128], MM_DT)
            ctx_sb = persist.tile([P, NPAIR, SEQ], MM_DT)
            # recip tiles (row 0 = 1/sums, rows 1..127 = 0 so the K=128
            # ones-matmul broadcast never multiplies 0 by inf/nan garbage)
            recipA = persist.tile([P, QH], F32)
            recipB = persist.tile([P, QH], F32)
            nc.vector.memset(recipA[:], 0.0)
            nc.vector.memset(recipB[:], 0.0)
            for h in range(HEADS):
                nc.vector.memset(v_sb[:, :, 128 * h], 1.0)

            # ---- emission helpers (all share the "s"/"c" psum slots) ----
            def q_chunk(m, nt):
                ps_t = ps.tile([P, QH], F32, tag="s", name=f"q{m}{nt}")
                for kk in range(KIN):
                    nc.tensor.matmul(
                        ps_t[:, 0:512],
                        wq_sb[:, kk, m * P:(m + 1) * P],
                        xq_sb[:, kk, nt * 512:(nt + 1) * 512],
                        start=(kk == 0), stop=(kk == KIN - 1),
                    )
                nc.vector.tensor_scalar_add(
                    qT_sb[:, m, nt * 512:(nt + 1) * 512], ps_t[:, 0:512],
                    qb_sb[:, m, None],
                )

            def k_chunk(m, isl):
                o0, w = ksl[isl]
                ps_t = ps.tile([P, QH], F32, tag="s", name=f"k{m}{isl}")
                for kk in range(KIN):
                    nc.tensor.matmul(
                        ps_t[:, 0:w],
                        wk_sb[:, kk, m * P:(m + 1) * P],
                        xk_sb[:, kk, o0:o0 + w],
                        start=(kk == 0), stop=(kk == KIN - 1),
                    )
                nc.vector.tensor_scalar_add(
                    kTz_sb[0:64, 2 * m, o0:o0 + w], ps_t[0:64, 0:w],
                    kb_sb[0:64, m, None],
                )
                nc.vector.tensor_scalar_add(
                    kTz_sb[64:128, 2 * m + 1, o0:o0 + w], ps_t[64:128, 0:w],
                    kb_sb[64:128, m, None],
                )

            def v_chunk(c):
                ps_t = ps.tile([P, QH], F32, tag="s", name=f"v{c}")
                for kk in range(KIN_V):
                    nc.tensor.matmul(
                        ps_t[:, 0:DGRP],
                        xv_sb[:, kk, c * P:(c + 1) * P],
                        wv_sb[:, kk, :],
                        start=(kk == 0), stop=(kk == KIN_V - 1),
                    )
                # strided evict into slots 64..127 of the 128-wide head lanes
                nc.vector.tensor_copy(
                    out=v_sb[:, c, :].rearrange("p (h x) -> p h x", x=128)[:, :, 64:128],
                    in_=ps_t[:, 0:DGRP].rearrange("p (h x) -> p h x", x=64),
                )

            def o_chunk(qc):
                ps_t = ps.tile([P, QH], F32, tag="s", name=f"o{qc}")
                for kk in range(3):
                    for n0, nsz in ((0, 512), (512, 256)):
                        nc.tensor.matmul(
                            ps_t[:, n0:n0 + nsz],
                            ctx_sb[:, kk, qc * P:(qc + 1) * P],
                            wo_sb[:, kk, n0:n0 + nsz],
                            start=(kk == 0), stop=(kk == 2),
                        )
                o_t = outp.tile([P, DIM], F32, tag="o", name=f"ot{qc}")
                nc.vector.tensor_tensor(
                    o_t[:], ps_t[:, 0:DIM], ob_bc[:], mybir.AluOpType.add,
                )
                nc.sync.dma_start(out[qc * P:(qc + 1) * P, :], o_t[:])

            def unit_tail_dve(pair, qh, cE, cO):
                # normalization tail, DVE piece: approx recip off psum row 0
                # (custom-DVE ops require partition-0 inputs), ctx dims
                # staged to SBUF (TensorTensor allows only one PSUM input).
                nc.vector.reciprocal_approx_fast(
                    out=recipA[0:1, :], in_=cE[0:1, :])
                nc.vector.reciprocal_approx_fast(
                    out=recipB[0:1, :], in_=cO[0:1, :])
                cuE = expp.tile([DH, QH], MM_DT, tag="ctxu", bufs=2,
                                name=f"cuE{qh}{pair}")
                nc.vector.tensor_copy(out=cuE[:], in_=cE[64:128, :])
                cuO = expp.tile([DH, QH], MM_DT, tag="ctxu", bufs=2,
                                name=f"cuO{qh}{pair}")
                nc.vector.tensor_copy(out=cuO[:], in_=cO[64:128, :])
                return cuE, cuO

            def unit_tail_fin(pair, qh, cE, cO, cuE, cuO):
                # normalization tail, PE piece: K=128 ones-matmul broadcast
                # of the recip into psum rows 0..63 (start=True resets), one
                # multiply into ctx_sb.  Emitted two kc slots after the DVE
                # piece so these matmuls never wait on the DVE chain.
                q0 = qh * QH
                for qt in range(2):
                    sl = slice(qt * 512, (qt + 1) * 512)
                    nc.tensor.matmul(cE[0:DH, sl], ones2_sb[:, 0:DH],
                                     recipA[:, sl], start=True, stop=True)
                    nc.tensor.matmul(cO[0:DH, sl], ones2_sb[:, 0:DH],
                                     recipB[:, sl], start=True, stop=True)
                nc.vector.tensor_tensor(
                    ctx_sb[0:DH, pair, q0:q0 + QH],
                    cuE[:], cE[0:DH, :], mybir.AluOpType.mult,
                )
                nc.vector.tensor_tensor(
                    ctx_sb[DH:P, pair, q0:q0 + QH],
                    cuO[:], cO[0:DH, :], mybir.AluOpType.mult,
                )

            # ---- fused attention + hidden projection/out-projection ----
            def ob_bcast(n0, nsz):
                bps = ps.tile([P, QH], F32, tag="s", name=f"bps{n0}")
                nc.tensor.matmul(bps[:, 0:nsz], ones2_sb[:],
                                 ob_row[:, n0:n0 + nsz], start=True, stop=True)
                nc.vector.tensor_copy(out=ob_bc[:, n0:n0 + nsz],
                                      in_=bps[:, 0:nsz])

            units = [(qh, pr) for qh in range(2) for pr in range(NPAIR)]
            hidden: dict = {
                (0, 0): [lambda c=c: v_chunk(c) for c in range(3, KC)]
                        + [lambda: q_chunk(1, 0), lambda: q_chunk(1, 1)]
                        + [lambda i=i: k_chunk(1, i) for i in range(len(ksl))],
                (0, 1): [lambda: ob_bcast(0, 512), lambda: ob_bcast(512, 256),
                         lambda: q_chunk(2, 0), lambda: q_chunk(2, 1)]
                        + [lambda i=i: k_chunk(2, i) for i in range(len(ksl))]
                        + [lambda: q_chunk(0, 2), lambda: q_chunk(0, 3)],
                (0, 2): [lambda: q_chunk(1, 2), lambda: q_chunk(1, 3),
                         lambda: q_chunk(2, 2), lambda: q_chunk(2, 3)],
                (1, 0): [lambda qc=qc: o_chunk(qc) for qc in range(0, 7)],
                (1, 1): [lambda: o_chunk(7)],
                (1, 2): [],
            }

            # lead-in: just enough projection to unblock unit (0, 0)
            q_chunk(0, 0)
            q_chunk(0, 1)
            for isl in range(len(ksl)):
                k_chunk(0, isl)
            for c in range(min(3, KC)):
                v_chunk(c)

            pending_fin = None
            for qh, pair in units:
                q0 = qh * QH
                work = list(hidden[(qh, pair)])
                cE = ps.tile([P, QH], F32, tag="c", name=f"cE{qh}{pair}")
                cO = ps.tile([P, QH], F32, tag="c", name=f"cO{qh}{pair}")
                weights = [None, None]  # per-parity weight views for ctx
                next_ctx = 0            # first weight-kc not yet ctx-matmul'd
                for kc in range(KC):
                    # scores per head (zero-padded K=128 contraction)
                    s_t = [None, None]
                    for par in range(2):
                        h = 2 * pair + par
                        st = ps.tile([P, QH], F32, tag="s",
                                     name=f"s{qh}{pair}{kc}{par}")
                        for qt in range(2):
                            nc.tensor.matmul(
                                st[:, qt * 512:(qt + 1) * 512],
                                kTz_sb[:, h, kc * P:(kc + 1) * P],
                                qT_sb[:, pair,
                                      q0 + qt * 512:q0 + (qt + 1) * 512],
                                start=True, stop=True,
                            )
                        s_t[par] = st
                    if kc == 2 and pending_fin is not None:
                        # PE piece of the previous unit's tail (its DVE
                        # chain, started at the previous unit's end, has
                        # drained by now).  This unit's first ctx matmuls
                        # must follow it: they reuse the "c" slots the tail
                        # reads, and the in-order PE queue cannot express
                        # the reverse order without deadlock.
                        pending_fin()
                        pending_fin = None
                    # ctx matmuls (leapfrog: PE never waits on the exp of
                    # the current kc; kc0/kc1 are additionally delayed past
                    # the previous unit's tail finisher above)
                    if kc >= 2:
                        for ckc in range(next_ctx, kc):
                            for par, cX in ((0, cE), (1, cO)):
                                h = 2 * pair + par
                                for qt in range(2):
                                    nc.tensor.matmul(
                                        cX[:, qt * 512:(qt + 1) * 512],
                                        v_sb[:, ckc, 128 * h:128 * h + 128],
                                        weights[par][ckc][
                                            :, qt * 512:(qt + 1) * 512],
                                        start=(ckc == 0), stop=False,
                                    )
                        next_ctx = kc
                    # exp: ACT or DVE (Schraudolph) by kv chunk
                    wts = [None, None]
                    for par in range(2):
                        if kc in dve_kc:
                            sch_t = schp.tile([P, QH], F32, tag="sch",
                                              bufs=5,
                                              name=f"sch{qh}{pair}{kc}{par}")
                            nc.vector.tensor_scalar(
                                sch_t[:], s_t[par][:],
                                SCH_SCALE, SCH_BIAS,
                                mybir.AluOpType.mult, mybir.AluOpType.add,
                            )
                            wts[par] = sch_t.bitcast(MM_DT).rearrange(
                                "p (n two) -> p n two", two=2)[:, :, 0]
                        else:
                            exp_t = expp.tile([P, QH], MM_DT, tag="exp",
                                              bufs=6,
                                              name=f"exp{qh}{pair}{kc}{par}")
                            nc.scalar.activation(
                                exp_t[:], s_t[par][:],
                                mybir.ActivationFunctionType.Exp,
                                bias=pb_sb[:, kc, None], scale=0.125,
                            )
                            wts[par] = exp_t
                    for par in range(2):
                        if weights[par] is None:
                            weights[par] = {}
                        weights[par][kc] = wts[par]
                    # hidden projection / out-projection chunks; pop two
                    # per slot when the list would otherwise not fit
                    if kc >= 3 and work:
                        work.pop(0)()
                        if len(work) > KC - 1 - kc:
                            work.pop(0)()
                # any hidden work that didn't fit in the kc slots (runs on
                # PE while ACT/DVE drain the exp backlog of this unit)
                while work:
                    work.pop(0)()
                # last ctx matmuls (kc = KC-1 weights), then the DVE tail
                for par, cX in ((0, cE), (1, cO)):
                    h = 2 * pair + par
                    for qt in range(2):
                        nc.tensor.matmul(
                            cX[:, qt * 512:(qt + 1) * 512],
                            v_sb[:, KC - 1, 128 * h:128 * h + 128],
                            weights[par][KC - 1][:, qt * 512:(qt + 1) * 512],
                            start=False, stop=True,
                        )
                cus = unit_tail_dve(pair, qh, cE, cO)
                pending_fin = (lambda pair=pair, qh=qh, cE=cE, cO=cO,
                               cus=cus:
                               unit_tail_fin(pair, qh, cE, cO, *cus))
            pending_fin()

            # out-projection for the second q half
            for qc in range(8, 16):
                o_chunk(qc)

    nc.compile()
    return nc


_cache: dict = {}

# test harnesses may set e.g. {"trace": True, "tmpdir": ...}; empty for grading
_run_opts: dict = {}
LAST_RES = None


def _get_nc(NKV: int):
    if NKV not in _cache:
        _cache[NKV] = _build(NKV)
    return _cache[NKV]


def kernel(query, key_, value, mask, q_w, q_b, k_w, k_b, v_w, v_b, o_w, o_b):
    query = np.asarray(query, np.float32)
    key_ = np.asarray(key_, np.float32)
    value = np.asarray(value, np.float32)
    mask = np.asarray(mask)
    q_w = np.asarray(q_w, np.float32)
    q_b = np.asarray(q_b, np.float32)
    k_w = np.asarray(k_w, np.float32)
    k_b = np.asarray(k_b, np.float32)
    v_w = np.asarray(v_w, np.float32)
    v_b = np.asarray(v_b, np.float32)
    o_w = np.asarray(o_w, np.float32)
    o_b = np.asarray(o_b, np.float32)

    counts = (mask != 0).sum(axis=1)
    NKV = max(P, int(-(-int(counts.max()) // P) * P))
    nc = _get_nc(NKV)

    zeros_ob = np.zeros_like(o_b)
    in_maps = []
    for b in range(BS):
        idx = np.nonzero(mask[b])[0]
        cnt = len(idx)
        xk_g = np.zeros((NKV, DIM), np.float32)
        xv_g = np.zeros((NKV, DIM), np.float32)
        xk_g[:cnt] = key_[b][idx]
        xv_g[:cnt] = value[b][idx]
        xqT_b = np.ascontiguousarray(query[b].T).astype(MM_NP)
        xkT_b = np.ascontiguousarray(xk_g.T).astype(MM_NP)
        xvT_b = np.zeros((P * KIN_V, NKV), MM_NP)
        xvT_b[:DIM] = xv_g.T
        xvT_b[DIM] = 1.0
        pb_b = np.where(np.arange(NKV) < cnt, BIAS_CENTER, NEG).astype(np.float32)
        for g in range(2):
            sl = slice(DGRP * g, DGRP * (g + 1))
            in_maps.append({
                "xqT": xqT_b,
                "xkT": xkT_b,
                "xvT": xvT_b,
                "wqT": np.ascontiguousarray(q_w[sl].T).astype(MM_NP),
                "wkT": np.ascontiguousarray(k_w[sl].T).astype(MM_NP),
                "wvT": np.concatenate(
                    [v_w[sl].T, v_b[None, sl],
                     np.zeros((P - 1, DGRP), np.float32)], axis=0).astype(MM_NP),
                "woT": np.ascontiguousarray(o_w[:, sl].T).astype(MM_NP),
                "qb": q_b[sl].copy(),
                "kb": k_b[sl].copy(),
                "ob": o_b if g == 0 else zeros_ob,
                "pb": pb_b,
            })

    res = run_bass_kernel_spmd(nc, in_maps, core_ids=list(range(N_CORES)),
                               **_run_opts)
    global LAST_RES
    LAST_RES = res
    out = np.empty((BS, SEQ, DIM), np.float32)
    for b in range(BS):
        out[b] = res.results[2 * b]["out"] + res.results[2 * b + 1]["out"]
    return out
